# revision 15
# baseline (speedup 1.0000x reference)
"""Trainium2 Bass kernel for nn_AttentionHead (B=4, T=4096, D=1024, H=64).

Sharding: 8 cores; core i handles (batch b = i//2, T-half = i%2): computes
attention output for its 2048 queries. K/V are computed per-core over the
full 4096 keys (weights tiny/replicated; key order is permutation-invariant
under softmax, so own-half-first ordering per core is fine).

Per-core dataflow (big matmuls in float32r = full-rate fp32 on the PE):
  - x tiles DMA'd contiguously, PE-transposed (fp32r transpose mode) to
    x^T staged in PSUM, copied to SBUF by DVE.
  - Projections with W^T stationary: k|v column-packed in one pass over
    x^T (k in one PSUM partition half, v in the other; halves swap for the
    second T-half so k lands where row-packed scores want it). relu+bias
    on ACT. q_T duplicated to partitions 64..127 via SBUF->SBUF DMA.
  - v_T PE-transposed to V natural [t,64]; column 64 = ones so attn@V also
    accumulates the softmax denominator.
  - scores s_T[k,q]: two k-tiles row-packed (contraction=64, row groups
    0/64) into one PSUM [128,1024] tile; exp on ACT with scale=1/8 (no max
    subtraction: scores are O(1) by construction). Two q-blocks are
    interleaved per k-pair so PE work hides the ACT exp chain.
  - attn@V: V'[128,65] stationary x exp[128,512] accumulated over 32
    k-tiles into PSUM [65,512]; row 64 = denominator. PE-transpose back,
    reciprocal*scale on DVE, DMA out.

Tensors are split at group granularity (kTp/Vg/qTb) so the Tile scheduler
can overlap the projection stage with attention as dependencies resolve.
"""

import os
import numpy as np

B, T, D, H = 4, 4096, 1024, 64
P = 128
NB = 512            # free-dim block size
TQ = T // 2         # queries per core
NCORES = 8

_cache = {}


def _build(use_f32r=True):
    import concourse.bass as bass
    import concourse.tile as tile
    from concourse import bacc, mybir
    from concourse.masks import make_identity

    f32 = mybir.dt.float32
    f32r = mybir.dt.float32r
    AF = mybir.ActivationFunctionType

    mmdt = f32r if use_f32r else f32

    nc = bacc.Bacc("TRN2", target_bir_lowering=False, debug=False)

    xa = nc.dram_tensor("xa", [TQ, D], f32, kind="ExternalInput").ap()
    xb = nc.dram_tensor("xb", [TQ, D], f32, kind="ExternalInput").ap()
    wqt = nc.dram_tensor("wqt", [D, H], f32, kind="ExternalInput").ap()
    wkt = nc.dram_tensor("wkt", [D, H], f32, kind="ExternalInput").ap()
    wvt = nc.dram_tensor("wvt", [D, H], f32, kind="ExternalInput").ap()
    bq = nc.dram_tensor("bq", [H, 1], f32, kind="ExternalInput").ap()
    bk = nc.dram_tensor("bk", [H, 1], f32, kind="ExternalInput").ap()
    bv = nc.dram_tensor("bv", [H, 1], f32, kind="ExternalInput").ap()
    out = nc.dram_tensor("o", [TQ, H], f32, kind="ExternalOutput").ap()

    NG = T // NB          # 8 K/V t-groups of 512
    NGH = NG // 2         # 4 groups per T-half
    NQB = TQ // NB        # 4 q-blocks of 512
    NKP = T // P // 2     # 16 k-tile pairs
    NC = D // P           # 8 d-chunks
    NJ = NB // P          # 4 t-subtiles per group

    with tile.TileContext(nc) as tc:
        with (
            tc.tile_pool(name="const", bufs=1) as constp,
            tc.tile_pool(name="persist", bufs=1) as persist,
            tc.tile_pool(name="xg", bufs=6) as xgp,
            tc.tile_pool(name="xt", bufs=8) as xtp,
            tc.tile_pool(name="kvsb", bufs=2) as kvp,
            tc.tile_pool(name="big_ps", bufs=2, space="PSUM") as bigp,
            tc.tile_pool(name="proj_ps", bufs=2, space="PSUM") as proj_psp,
            tc.tile_pool(name="o_ps", bufs=2, space="PSUM") as o_psp,
            tc.tile_pool(name="esb", bufs=4) as esbp,
            tc.tile_pool(name="osb", bufs=2) as osbp,
            tc.tile_pool(name="outp", bufs=2) as outp,
        ):
            ident = constp.tile([P, P], f32)
            make_identity(nc, ident)
            ident_r = constp.tile([P, P], mmdt)
            nc.vector.tensor_copy(ident_r, ident)
            wq_ld = constp.tile([P, NC, H], f32)
            wk_ld = constp.tile([P, NC, H], f32)
            wv_ld = constp.tile([P, NC, H], f32)
            wq_sb = constp.tile([P, NC, H], mmdt)
            wk_sb = constp.tile([P, NC, H], mmdt)
            wv_sb = constp.tile([P, NC, H], mmdt)
            bq_sb = constp.tile([H, 1], f32)
            bk_sb = constp.tile([H, 1], f32)
            bv_sb = constp.tile([H, 1], f32)

            def load_consts():
                # emitted after the first x-block DMA so the transposes (the
                # first PE work) aren't stuck behind the weight loads; DVE
                # copies round fp32 -> fp32r (walrus requires rounded inputs)
                nc.sync.dma_start(wq_ld, wqt.rearrange("(c p) h -> p c h", p=P))
                nc.sync.dma_start(wk_ld, wkt.rearrange("(c p) h -> p c h", p=P))
                nc.sync.dma_start(wv_ld, wvt.rearrange("(c p) h -> p c h", p=P))
                nc.vector.tensor_copy(wq_sb, wq_ld)
                nc.vector.tensor_copy(wk_sb, wk_ld)
                nc.vector.tensor_copy(wv_sb, wv_ld)
                nc.sync.dma_start(bq_sb, bq)
                nc.sync.dma_start(bk_sb, bk)
                nc.sync.dma_start(bv_sb, bv)

            # persistent attention operands, split per group for overlap
            kTp = [persist.tile([P, NJ, P], mmdt, name=f"kTp{j}") for j in range(NGH)]
            qTb = [persist.tile([P, NB], mmdt, name=f"qTb{j}") for j in range(NQB)]
            Vg = [persist.tile([P, NJ, H + 1], mmdt, name=f"Vg{g}") for g in range(NG)]
            onesc = constp.tile([P, NJ, 1], f32)
            nc.gpsimd.memset(onesc, 1.0)
            for g in range(NG):
                nc.vector.tensor_copy(Vg[g][:, :, H : H + 1], onesc)

            # ---------------- projections ----------------
            def do_group(g, after_dma=None):
                half2 = g >= NGH          # second T-half (keys 2048..4095)
                src = xb if half2 else xa
                j = g % NGH
                r0 = j * NB
                # two half-loads so transposes start after the first lands
                xga = xgp.tile([P, NJ // 2, D], f32, tag="xg")
                xgb = xgp.tile([P, NJ // 2, D], f32, tag="xg")
                nc.sync.dma_start(
                    xga,
                    src[r0 : r0 + NB // 2, :].rearrange("(j p) d -> p j d", p=P),
                )
                nc.sync.dma_start(
                    xgb,
                    src[r0 + NB // 2 : r0 + NB, :].rearrange("(j p) d -> p j d", p=P),
                )
                if after_dma is not None:
                    after_dma()

                def xg(jj):
                    return (xga if jj < NJ // 2 else xgb)[:, jj % (NJ // 2), :]
                # transpose to x^T: c-pairs staged through one [128,1024] bank-pair
                xts = []
                for cp in range(NC // 2):
                    pt = bigp.tile([P, 2 * NB], f32, tag="big")
                    for ci in range(2):
                        c = 2 * cp + ci
                        for jj in range(NJ):
                            nc.tensor.transpose(
                                pt[:, ci * NB + jj * P : ci * NB + (jj + 1) * P],
                                xg(jj)[:, c * P : (c + 1) * P],
                                ident,
                            )
                    xt = xtp.tile([P, 2, NB], mmdt)
                    nc.vector.tensor_copy(xt, pt.rearrange("p (c n) -> p c n", c=2))
                    xts.append(xt)

                # K and V projections (separate passes: fp32r forbids
                # column tile_position packing). Both land at partitions 0:64.
                k_ps = proj_psp.tile([H, NB], f32, tag="proj", name="k_ps")
                for c in range(NC):
                    nc.tensor.matmul(
                        k_ps,
                        wk_sb[:, c, :],
                        xts[c // 2][:, c % 2, :],
                        start=(c == 0),
                        stop=(c == NC - 1),
                    )
                k_sb = kvp.tile([H, NB], mmdt, tag="kvsb", name="k_sb")
                nc.scalar.activation(k_sb, k_ps, AF.Relu, bias=bk_sb[:, 0:1])
                # first T-half k -> partitions 0:64 of kTp (DVE, lane-aligned);
                # second T-half k -> partitions 64:128 (SBUF->SBUF DMA shift)
                if not half2:
                    nc.vector.tensor_copy(
                        kTp[j][0:H, :, :],
                        k_sb.rearrange("h (j t) -> h j t", t=P),
                    )
                else:
                    nc.sync.dma_start(
                        kTp[j][H:P, :, :],
                        k_sb.rearrange("h (j t) -> h j t", t=P),
                    )

                v_ps = proj_psp.tile([H, NB], f32, tag="proj", name="v_ps")
                for c in range(NC):
                    nc.tensor.matmul(
                        v_ps,
                        wv_sb[:, c, :],
                        xts[c // 2][:, c % 2, :],
                        start=(c == 0),
                        stop=(c == NC - 1),
                    )
                v_sb = kvp.tile([H, NB], mmdt, tag="kvsb", name="v_sb")
                nc.scalar.activation(v_sb, v_ps, AF.Relu, bias=bv_sb[:, 0:1])
                vt_ps = proj_psp.tile([P, NJ, H], mmdt, tag="proj", name="vt_ps")
                for jj in range(NJ):
                    nc.tensor.transpose(
                        vt_ps[:, jj, :],
                        v_sb[:, jj * P : (jj + 1) * P],
                        ident_r[0:H, 0:H],
                    )
                nc.vector.tensor_copy(Vg[g][:, :, 0:H], vt_ps)

                # Q projection for own half
                if not half2:
                    q_ps = proj_psp.tile([H, NB], f32, tag="proj", name="q_ps")
                    for c in range(NC):
                        nc.tensor.matmul(
                            q_ps,
                            wq_sb[:, c, :],
                            xts[c // 2][:, c % 2, :],
                            start=(c == 0),
                            stop=(c == NC - 1),
                        )
                    nc.scalar.activation(
                        qTb[j][0:H, :], q_ps, AF.Relu, bias=bq_sb[:, 0:1]
                    )
                    nc.sync.dma_start(qTb[j][H:P, :], qTb[j][0:H, :])

            for j in range(NGH):
                do_group(j, after_dma=load_consts if j == 0 else None)
                do_group(j + NGH)

            # ---------------- attention ----------------
            scale = float(1.0 / np.sqrt(H))
            for qbp in range(NQB // 2):
                qbs = (2 * qbp, 2 * qbp + 1)
                o_ps = {
                    qb: o_psp.tile([H + 1, NB], f32, name=f"o_ps{qb}", tag="o_ps")
                    for qb in qbs
                }
                for p in range(NKP):
                    jg, i = p // NJ, p % NJ
                    e2 = {}
                    for qb in qbs:
                        s2 = bigp.tile([P, 2 * NB], f32, tag="big")
                        nc.tensor.matmul(
                            s2[:, 0:NB],
                            kTp[jg][0:H, i, :],
                            qTb[qb][0:H, :],
                            start=True,
                            stop=True,
                            tile_position=(0, 0),
                        )
                        nc.tensor.matmul(
                            s2[:, NB : 2 * NB],
                            kTp[jg][H:P, i, :],
                            qTb[qb][H:P, :],
                            start=True,
                            stop=True,
                            tile_position=(H, 0),
                        )
                        e = esbp.tile([P, 2 * NB], mmdt)
                        nc.scalar.activation(e, s2, AF.Exp, scale=scale)
                        e2[qb] = e
                    for qb in qbs:
                        nc.tensor.matmul(
                            o_ps[qb],
                            Vg[jg][:, i, :],
                            e2[qb][:, 0:NB],
                            start=(p == 0),
                            stop=False,
                        )
                        nc.tensor.matmul(
                            o_ps[qb],
                            Vg[NGH + jg][:, i, :],
                            e2[qb][:, NB : 2 * NB],
                            start=False,
                            stop=(p == NKP - 1),
                        )
                # normalize and store
                for qb in qbs:
                    o_sb = osbp.tile([H + 1, NB], f32)
                    nc.vector.tensor_copy(o_sb, o_ps[qb])
                    o4 = outp.tile([P, NJ, H], f32)
                    for jj in range(NJ):
                        ot = bigp.tile([P, H + 1], f32, tag="big")
                        nc.tensor.transpose(
                            ot,
                            o_sb[:, jj * P : (jj + 1) * P],
                            ident[0 : H + 1, 0 : H + 1],
                        )
                        recip = osbp.tile([P, 1], f32, tag="recip")
                        nc.vector.reciprocal(recip, ot[:, H : H + 1])
                        nc.vector.tensor_scalar_mul(o4[:, jj, :], ot[:, 0:H], recip)
                    q0 = qb * NB
                    nc.sync.dma_start(
                        out[q0 : q0 + NB, :].rearrange("(j p) h -> p j h", p=P), o4
                    )

    nc.compile()
    return nc


def _get_nc():
    if "nc" not in _cache:
        _cache["nc"] = _build(use_f32r=os.environ.get("K_NO_F32R", "") != "1")
    return _cache["nc"]


def _prep_inputs(x, Wk, bk, Wq, bq, Wv, bv):
    x = np.asarray(x, np.float32)
    wqt = np.ascontiguousarray(np.asarray(Wq, np.float32).T)
    wkt = np.ascontiguousarray(np.asarray(Wk, np.float32).T)
    wvt = np.ascontiguousarray(np.asarray(Wv, np.float32).T)
    bqc = np.asarray(bq, np.float32).reshape(H, 1)
    bkc = np.asarray(bk, np.float32).reshape(H, 1)
    bvc = np.asarray(bv, np.float32).reshape(H, 1)
    in_maps = []
    for i in range(NCORES):
        b, h = i // 2, i % 2
        xa = np.ascontiguousarray(x[b, h * TQ : (h + 1) * TQ])
        xbo = np.ascontiguousarray(x[b, (1 - h) * TQ : (2 - h) * TQ])
        in_maps.append(
            dict(xa=xa, xb=xbo, wqt=wqt, wkt=wkt, wvt=wvt,
                 bq=bqc, bk=bkc, bv=bvc)
        )
    return in_maps


def run(inputs, trace=False):
    from concourse.bass_utils import run_bass_kernel_spmd

    nc = _get_nc()
    in_maps = _prep_inputs(**inputs)
    res = run_bass_kernel_spmd(nc, in_maps, list(range(NCORES)), trace=trace)
    full = np.empty((B, T, H), np.float32)
    for i in range(NCORES):
        b, h = i // 2, i % 2
        full[b, h * TQ : (h + 1) * TQ] = res.results[i]["o"]
    return full, res


def kernel(**inputs):
    out, _ = run(inputs, trace=False)
    return out


# revision 17
# speedup vs baseline: 5.3669x; 5.3669x over previous
"""Trainium2 Bass kernel for nn_AttentionHead (B=4, T=4096, D=1024, H=64).

Sharding: 8 cores; core i handles (batch b = i//2, T-half = i%2): computes
attention output for its 2048 queries. K/V are computed per-core over the
full 4096 keys (weights tiny/replicated; key order is permutation-invariant
under softmax, so own-half-first ordering per core is fine).

Per-core dataflow (big matmuls in float32r = full-rate fp32 on the PE;
walrus requires fp32r operands to be produced *rounded*, so every matmul
input comes from a DVE copy or ACT activation with fp32r output dtype):
  - x tiles DMA'd contiguously, PE-transposed in fp32 to x^T staged in
    PSUM; the DVE copy to SBUF is the fp32->fp32r rounding point.
  - Projections with W^T stationary produce k_T/q_T [h, t] and v_T.
    relu+bias on ACT (fp32r out). First-half k goes to kTp partitions
    0:64 (DVE copy); second-half k to partitions 64:128 via SBUF->SBUF
    DMA shift. q_T duplicated to partitions 64:128 the same way, so
    scores can row-pack two 64-contraction matmuls per PSUM tile.
  - v_T PE-transposed to V natural [t,64]; column 64 = ones so attn@V also
    accumulates the softmax denominator.
  - scores s_T[k,q]: two k-tiles row-packed (contraction=64, row groups
    0/64) into one PSUM [128,1024] tile; exp on ACT with scale=1/8 (no max
    subtraction: scores are O(1) by construction). Two q-blocks are
    interleaved per k-pair so PE work hides the ACT exp chain.
  - attn@V: V'[128,65] stationary x exp[128,512] accumulated over 32
    k-tiles into PSUM [65,512]; row 64 = denominator. PE-transpose back,
    reciprocal*scale on DVE, DMA out.

Tensors are split at group granularity (kTp/Vg/qTb) so the Tile scheduler
can overlap the projection stage with attention as dependencies resolve.
"""

import os
import numpy as np

B, T, D, H = 4, 4096, 1024, 64
P = 128
NB = 512            # free-dim block size
TQ = T // 2         # queries per core
NCORES = 8

_cache = {}


def _build(use_f32r=True):
    import concourse.bass as bass
    import concourse.tile as tile
    from concourse import bacc, mybir
    from concourse.masks import make_identity

    f32 = mybir.dt.float32
    f32r = mybir.dt.float32r
    AF = mybir.ActivationFunctionType

    mmdt = f32r if use_f32r else f32

    nc = bacc.Bacc("TRN2", target_bir_lowering=False, debug=False)

    xa = nc.dram_tensor("xa", [TQ, D], f32, kind="ExternalInput").ap()
    xb = nc.dram_tensor("xb", [TQ, D], f32, kind="ExternalInput").ap()
    wqt = nc.dram_tensor("wqt", [D, H], f32, kind="ExternalInput").ap()
    wkt = nc.dram_tensor("wkt", [D, H], f32, kind="ExternalInput").ap()
    wvt = nc.dram_tensor("wvt", [D, H], f32, kind="ExternalInput").ap()
    bq = nc.dram_tensor("bq", [H, 1], f32, kind="ExternalInput").ap()
    bk = nc.dram_tensor("bk", [H, 1], f32, kind="ExternalInput").ap()
    bv = nc.dram_tensor("bv", [H, 1], f32, kind="ExternalInput").ap()
    out = nc.dram_tensor("o", [TQ, H], f32, kind="ExternalOutput").ap()

    NG = T // NB          # 8 K/V t-groups of 512
    NGH = NG // 2         # 4 groups per T-half
    NQB = TQ // NB        # 4 q-blocks of 512
    NKP = T // P // 2     # 16 k-tile pairs
    NC = D // P           # 8 d-chunks
    NJ = NB // P          # 4 t-subtiles per group

    with tile.TileContext(nc) as tc:
        with (
            tc.tile_pool(name="const", bufs=1) as constp,
            tc.tile_pool(name="persist", bufs=1) as persist,
            tc.tile_pool(name="xg", bufs=6) as xgp,
            tc.tile_pool(name="xt", bufs=8) as xtp,
            tc.tile_pool(name="kvsb", bufs=2) as kvp,
            tc.tile_pool(name="big_ps", bufs=2, space="PSUM") as bigp,
            tc.tile_pool(name="proj_ps", bufs=2, space="PSUM") as proj_psp,
            tc.tile_pool(name="o_ps", bufs=2, space="PSUM") as o_psp,
            tc.tile_pool(name="esb", bufs=4) as esbp,
            tc.tile_pool(name="osb", bufs=2) as osbp,
            tc.tile_pool(name="outp", bufs=2) as outp,
        ):
            ident = constp.tile([P, P], f32)
            make_identity(nc, ident)
            ident_r = constp.tile([P, P], mmdt)
            nc.vector.tensor_copy(ident_r, ident)
            wq_ld = constp.tile([P, NC, H], f32)
            wk_ld = constp.tile([P, NC, H], f32)
            wv_ld = constp.tile([P, NC, H], f32)
            wq_sb = constp.tile([P, NC, H], mmdt)
            wk_sb = constp.tile([P, NC, H], mmdt)
            wv_sb = constp.tile([P, NC, H], mmdt)
            bq_sb = constp.tile([H, 1], f32)
            bk_sb = constp.tile([H, 1], f32)
            bv_sb = constp.tile([H, 1], f32)

            def load_consts():
                # emitted after the first x-block DMA so the transposes (the
                # first PE work) aren't stuck behind the weight loads; DVE
                # copies round fp32 -> fp32r (walrus requires rounded inputs)
                nc.sync.dma_start(wq_ld, wqt.rearrange("(c p) h -> p c h", p=P))
                nc.sync.dma_start(wk_ld, wkt.rearrange("(c p) h -> p c h", p=P))
                nc.sync.dma_start(wv_ld, wvt.rearrange("(c p) h -> p c h", p=P))
                nc.vector.tensor_copy(wq_sb, wq_ld)
                nc.vector.tensor_copy(wk_sb, wk_ld)
                nc.vector.tensor_copy(wv_sb, wv_ld)
                nc.sync.dma_start(bq_sb, bq)
                nc.sync.dma_start(bk_sb, bk)
                nc.sync.dma_start(bv_sb, bv)

            # persistent attention operands, split per group for overlap
            kTp = [persist.tile([P, NJ, P], mmdt, name=f"kTp{j}") for j in range(NGH)]
            qTb = [persist.tile([P, NB], mmdt, name=f"qTb{j}") for j in range(NQB)]
            Vg = [persist.tile([P, NJ, H + 1], mmdt, name=f"Vg{g}") for g in range(NG)]
            onesc = constp.tile([P, NJ, 1], f32)
            nc.gpsimd.memset(onesc, 1.0)
            for g in range(NG):
                nc.vector.tensor_copy(Vg[g][:, :, H : H + 1], onesc)

            # ---------------- projections ----------------
            def do_group(g, after_dma=None):
                half2 = g >= NGH          # second T-half (keys 2048..4095)
                src = xb if half2 else xa
                j = g % NGH
                r0 = j * NB
                # two half-loads so transposes start after the first lands
                xga = xgp.tile([P, NJ // 2, D], f32, tag="xg")
                xgb = xgp.tile([P, NJ // 2, D], f32, tag="xg")
                nc.sync.dma_start(
                    xga,
                    src[r0 : r0 + NB // 2, :].rearrange("(j p) d -> p j d", p=P),
                )
                nc.sync.dma_start(
                    xgb,
                    src[r0 + NB // 2 : r0 + NB, :].rearrange("(j p) d -> p j d", p=P),
                )
                if after_dma is not None:
                    after_dma()

                def xg(jj):
                    return (xga if jj < NJ // 2 else xgb)[:, jj % (NJ // 2), :]
                # transpose to x^T: c-pairs staged through one [128,1024] bank-pair
                xts = []
                for cp in range(NC // 2):
                    pt = bigp.tile([P, 2 * NB], f32, tag="big")
                    for ci in range(2):
                        c = 2 * cp + ci
                        for jj in range(NJ):
                            nc.tensor.transpose(
                                pt[:, ci * NB + jj * P : ci * NB + (jj + 1) * P],
                                xg(jj)[:, c * P : (c + 1) * P],
                                ident,
                            )
                    xt = xtp.tile([P, 2, NB], mmdt)
                    nc.vector.tensor_copy(xt, pt.rearrange("p (c n) -> p c n", c=2))
                    xts.append(xt)

                # K and V projections (separate passes: fp32r forbids
                # column tile_position packing). Both land at partitions 0:64.
                k_ps = proj_psp.tile([H, NB], f32, tag="proj", name="k_ps")
                for c in range(NC):
                    nc.tensor.matmul(
                        k_ps,
                        wk_sb[:, c, :],
                        xts[c // 2][:, c % 2, :],
                        start=(c == 0),
                        stop=(c == NC - 1),
                    )
                k_sb = kvp.tile([H, NB], mmdt, tag="kvsb", name="k_sb")
                nc.scalar.activation(k_sb, k_ps, AF.Relu, bias=bk_sb[:, 0:1])
                # first T-half k -> partitions 0:64 of kTp (DVE, lane-aligned);
                # second T-half k -> partitions 64:128 (SBUF->SBUF DMA shift)
                if not half2:
                    nc.vector.tensor_copy(
                        kTp[j][0:H, :, :],
                        k_sb.rearrange("h (j t) -> h j t", t=P),
                    )
                else:
                    nc.sync.dma_start(
                        kTp[j][H:P, :, :],
                        k_sb.rearrange("h (j t) -> h j t", t=P),
                    )

                v_ps = proj_psp.tile([H, NB], f32, tag="proj", name="v_ps")
                for c in range(NC):
                    nc.tensor.matmul(
                        v_ps,
                        wv_sb[:, c, :],
                        xts[c // 2][:, c % 2, :],
                        start=(c == 0),
                        stop=(c == NC - 1),
                    )
                v_sb = kvp.tile([H, NB], mmdt, tag="kvsb", name="v_sb")
                nc.scalar.activation(v_sb, v_ps, AF.Relu, bias=bv_sb[:, 0:1])
                vt_ps = proj_psp.tile([P, NJ, H], mmdt, tag="proj", name="vt_ps")
                for jj in range(NJ):
                    nc.tensor.transpose(
                        vt_ps[:, jj, :],
                        v_sb[:, jj * P : (jj + 1) * P],
                        ident_r[0:H, 0:H],
                    )
                nc.vector.tensor_copy(Vg[g][:, :, 0:H], vt_ps)

                # Q projection for own half
                if not half2:
                    q_ps = proj_psp.tile([H, NB], f32, tag="proj", name="q_ps")
                    for c in range(NC):
                        nc.tensor.matmul(
                            q_ps,
                            wq_sb[:, c, :],
                            xts[c // 2][:, c % 2, :],
                            start=(c == 0),
                            stop=(c == NC - 1),
                        )
                    nc.scalar.activation(
                        qTb[j][0:H, :], q_ps, AF.Relu, bias=bq_sb[:, 0:1]
                    )
                    nc.sync.dma_start(qTb[j][H:P, :], qTb[j][0:H, :])

            for j in range(NGH):
                do_group(j, after_dma=load_consts if j == 0 else None)
                do_group(j + NGH)

            # ---------------- attention ----------------
            scale = float(1.0 / np.sqrt(H))
            for qbp in range(NQB // 2):
                qbs = (2 * qbp, 2 * qbp + 1)
                o_ps = {
                    qb: o_psp.tile([H + 1, NB], f32, name=f"o_ps{qb}", tag="o_ps")
                    for qb in qbs
                }
                for p in range(NKP):
                    jg, i = p // NJ, p % NJ
                    e2 = {}
                    for qb in qbs:
                        s2 = bigp.tile([P, 2 * NB], f32, tag="big")
                        nc.tensor.matmul(
                            s2[:, 0:NB],
                            kTp[jg][0:H, i, :],
                            qTb[qb][0:H, :],
                            start=True,
                            stop=True,
                            tile_position=(0, 0),
                        )
                        nc.tensor.matmul(
                            s2[:, NB : 2 * NB],
                            kTp[jg][H:P, i, :],
                            qTb[qb][H:P, :],
                            start=True,
                            stop=True,
                            tile_position=(H, 0),
                        )
                        e = esbp.tile([P, 2 * NB], mmdt)
                        nc.scalar.activation(e, s2, AF.Exp, scale=scale)
                        e2[qb] = e
                    for qb in qbs:
                        nc.tensor.matmul(
                            o_ps[qb],
                            Vg[jg][:, i, :],
                            e2[qb][:, 0:NB],
                            start=(p == 0),
                            stop=False,
                        )
                        nc.tensor.matmul(
                            o_ps[qb],
                            Vg[NGH + jg][:, i, :],
                            e2[qb][:, NB : 2 * NB],
                            start=False,
                            stop=(p == NKP - 1),
                        )
                # normalize and store
                for qb in qbs:
                    o_sb = osbp.tile([H + 1, NB], f32)
                    nc.vector.tensor_copy(o_sb, o_ps[qb])
                    o4 = outp.tile([P, NJ, H], f32)
                    for jj in range(NJ):
                        ot = bigp.tile([P, H + 1], f32, tag="big")
                        nc.tensor.transpose(
                            ot,
                            o_sb[:, jj * P : (jj + 1) * P],
                            ident[0 : H + 1, 0 : H + 1],
                        )
                        recip = osbp.tile([P, 1], f32, tag="recip")
                        nc.vector.reciprocal(recip, ot[:, H : H + 1])
                        nc.vector.tensor_scalar_mul(o4[:, jj, :], ot[:, 0:H], recip)
                    q0 = qb * NB
                    nc.sync.dma_start(
                        out[q0 : q0 + NB, :].rearrange("(j p) h -> p j h", p=P), o4
                    )

    nc.compile()
    return nc


def _get_nc():
    if "nc" not in _cache:
        _cache["nc"] = _build(use_f32r=os.environ.get("K_NO_F32R", "") != "1")
    return _cache["nc"]


def _prep_inputs(x, Wk, bk, Wq, bq, Wv, bv):
    x = np.asarray(x, np.float32)
    wqt = np.ascontiguousarray(np.asarray(Wq, np.float32).T)
    wkt = np.ascontiguousarray(np.asarray(Wk, np.float32).T)
    wvt = np.ascontiguousarray(np.asarray(Wv, np.float32).T)
    bqc = np.asarray(bq, np.float32).reshape(H, 1)
    bkc = np.asarray(bk, np.float32).reshape(H, 1)
    bvc = np.asarray(bv, np.float32).reshape(H, 1)
    in_maps = []
    for i in range(NCORES):
        b, h = i // 2, i % 2
        xa = np.ascontiguousarray(x[b, h * TQ : (h + 1) * TQ])
        xbo = np.ascontiguousarray(x[b, (1 - h) * TQ : (2 - h) * TQ])
        in_maps.append(
            dict(xa=xa, xb=xbo, wqt=wqt, wkt=wkt, wvt=wvt,
                 bq=bqc, bk=bkc, bv=bvc)
        )
    return in_maps


def run(inputs, trace=False):
    from concourse.bass_utils import run_bass_kernel_spmd

    if not trace:
        # NTFF profiling is unavailable in this environment; make sure an
        # ambient BASS_TRACE can't divert the execute path.
        os.environ["BASS_NEVER_TRACE"] = "1"
    nc = _get_nc()
    in_maps = _prep_inputs(**inputs)
    res = run_bass_kernel_spmd(nc, in_maps, list(range(NCORES)), trace=trace)
    full = np.empty((B, T, H), np.float32)
    for i in range(NCORES):
        b, h = i // 2, i % 2
        full[b, h * TQ : (h + 1) * TQ] = res.results[i]["o"]
    return full, res


def kernel(**inputs):
    out, _ = run(inputs, trace=False)
    return out


# revision 18
# speedup vs baseline: 5.6788x; 1.0581x over previous
"""Trainium2 Bass kernel for nn_AttentionHead (B=4, T=4096, D=1024, H=64).

Sharding: 8 cores; core i handles (batch b = i//2, T-half = i%2): computes
attention output for its 2048 queries. K/V are computed per-core over the
full 4096 keys (weights tiny/replicated; key order is permutation-invariant
under softmax, so own-half-first ordering per core is fine).

Per-core dataflow (big matmuls in float32r = full-rate fp32 on the PE;
walrus requires fp32r operands to be produced *rounded*, so every matmul
input comes from a DVE copy or ACT activation with fp32r output dtype):
  - x tiles DMA'd contiguously, PE-transposed in fp32 to x^T staged in
    PSUM; the DVE copy to SBUF is the fp32->fp32r rounding point.
  - Projections with W^T stationary produce k_T/q_T [h, t] and v_T.
    relu+bias on ACT (fp32r out). First-half k goes to kTp partitions
    0:64 (DVE copy); second-half k to partitions 64:128 via SBUF->SBUF
    DMA shift. q_T duplicated to partitions 64:128 the same way, so
    scores can row-pack two 64-contraction matmuls per PSUM tile.
  - v_T PE-transposed to V natural [t,64]; column 64 = ones so attn@V also
    accumulates the softmax denominator.
  - scores s_T[k,q]: two k-tiles row-packed (contraction=64, row groups
    0/64) into one PSUM [128,1024] tile; exp on ACT with scale=1/8 (no max
    subtraction: scores are O(1) by construction). Two q-blocks are
    interleaved per k-pair so PE work hides the ACT exp chain.
  - attn@V: V'[128,65] stationary x exp[128,512] accumulated over 32
    k-tiles into PSUM [65,512]; row 64 = denominator. PE-transpose back,
    reciprocal*scale on DVE, DMA out.

Tensors are split at group granularity (kTp/Vg/qTb) so the Tile scheduler
can overlap the projection stage with attention as dependencies resolve.
"""

import os
import numpy as np

B, T, D, H = 4, 4096, 1024, 64
P = 128
NB = 512            # free-dim block size
TQ = T // 2         # queries per core
NCORES = 8

_cache = {}


def _build(use_f32r=True):
    import concourse.bass as bass
    import concourse.tile as tile
    from concourse import bacc, mybir
    from concourse.masks import make_identity

    f32 = mybir.dt.float32
    f32r = mybir.dt.float32r
    AF = mybir.ActivationFunctionType

    mmdt = f32r if use_f32r else f32

    nc = bacc.Bacc("TRN2", target_bir_lowering=False, debug=False)

    xa = nc.dram_tensor("xa", [TQ, D], f32, kind="ExternalInput").ap()
    xb = nc.dram_tensor("xb", [TQ, D], f32, kind="ExternalInput").ap()
    wkq = nc.dram_tensor("wkq", [D, P], f32, kind="ExternalInput").ap()
    wvk = nc.dram_tensor("wvk", [D, P], f32, kind="ExternalInput").ap()
    wvt = nc.dram_tensor("wvt", [D, H], f32, kind="ExternalInput").ap()
    bkq = nc.dram_tensor("bkq", [P, 1], f32, kind="ExternalInput").ap()
    bvk = nc.dram_tensor("bvk", [P, 1], f32, kind="ExternalInput").ap()
    bv = nc.dram_tensor("bv", [H, 1], f32, kind="ExternalInput").ap()
    out = nc.dram_tensor("o", [TQ, H], f32, kind="ExternalOutput").ap()

    NG = T // NB          # 8 K/V t-groups of 512
    NGH = NG // 2         # 4 groups per T-half
    NQB = TQ // NB        # 4 q-blocks of 512
    NKP = T // P // 2     # 16 k-tile pairs
    NC = D // P           # 8 d-chunks
    NJ = NB // P          # 4 t-subtiles per group

    with tile.TileContext(nc) as tc:
        with (
            tc.tile_pool(name="const", bufs=1) as constp,
            tc.tile_pool(name="persist", bufs=1) as persist,
            tc.tile_pool(name="xg", bufs=6) as xgp,
            tc.tile_pool(name="xt", bufs=8) as xtp,
            tc.tile_pool(name="kvsb", bufs=2) as kvp,
            tc.tile_pool(name="big_ps", bufs=2, space="PSUM") as bigp,
            tc.tile_pool(name="proj_ps", bufs=2, space="PSUM") as proj_psp,
            tc.tile_pool(name="o_ps", bufs=2, space="PSUM") as o_psp,
            tc.tile_pool(name="esb", bufs=4) as esbp,
            tc.tile_pool(name="osb", bufs=2) as osbp,
            tc.tile_pool(name="outp", bufs=2) as outp,
        ):
            ident = constp.tile([P, P], f32)
            make_identity(nc, ident)
            ident_r = constp.tile([P, P], mmdt)
            nc.vector.tensor_copy(ident_r, ident)
            wkq_ld = constp.tile([P, NC, P], f32)
            wvk_ld = constp.tile([P, NC, P], f32)
            wv_ld = constp.tile([P, NC, H], f32)
            wkq_sb = constp.tile([P, NC, P], mmdt)
            wvk_sb = constp.tile([P, NC, P], mmdt)
            wv_sb = constp.tile([P, NC, H], mmdt)
            bkq_sb = constp.tile([P, 1], f32)
            bvk_sb = constp.tile([P, 1], f32)
            bv_sb = constp.tile([H, 1], f32)

            def load_consts():
                # emitted after the first x-block DMA so the transposes (the
                # first PE work) aren't stuck behind the weight loads; DVE
                # copies round fp32 -> fp32r (walrus requires rounded inputs)
                nc.sync.dma_start(wkq_ld, wkq.rearrange("(c p) h -> p c h", p=P))
                nc.sync.dma_start(wvk_ld, wvk.rearrange("(c p) h -> p c h", p=P))
                nc.sync.dma_start(wv_ld, wvt.rearrange("(c p) h -> p c h", p=P))
                nc.vector.tensor_copy(wkq_sb, wkq_ld)
                nc.vector.tensor_copy(wvk_sb, wvk_ld)
                nc.vector.tensor_copy(wv_sb, wv_ld)
                nc.sync.dma_start(bkq_sb, bkq)
                nc.sync.dma_start(bvk_sb, bvk)
                nc.sync.dma_start(bv_sb, bv)

            # persistent attention operands, split per group for overlap
            kTp = [persist.tile([P, NJ, P], mmdt, name=f"kTp{j}") for j in range(NGH)]
            qTb = [persist.tile([P, NB], mmdt, name=f"qTb{j}") for j in range(NQB)]
            Vg = [persist.tile([P, NJ, H + 1], mmdt, name=f"Vg{g}") for g in range(NG)]
            onesc = constp.tile([P, NJ, 1], f32)
            nc.gpsimd.memset(onesc, 1.0)
            for g in range(NG):
                nc.vector.tensor_copy(Vg[g][:, :, H : H + 1], onesc)

            # ---------------- projections ----------------
            def do_group(g, after_dma=None):
                half2 = g >= NGH          # second T-half (keys 2048..4095)
                src = xb if half2 else xa
                j = g % NGH
                r0 = j * NB
                # two half-loads so transposes start after the first lands
                xga = xgp.tile([P, NJ // 2, D], f32, tag="xg")
                xgb = xgp.tile([P, NJ // 2, D], f32, tag="xg")
                nc.sync.dma_start(
                    xga,
                    src[r0 : r0 + NB // 2, :].rearrange("(j p) d -> p j d", p=P),
                )
                nc.sync.dma_start(
                    xgb,
                    src[r0 + NB // 2 : r0 + NB, :].rearrange("(j p) d -> p j d", p=P),
                )
                if after_dma is not None:
                    after_dma()

                def xg(jj):
                    return (xga if jj < NJ // 2 else xgb)[:, jj % (NJ // 2), :]
                # transpose to x^T: c-pairs staged through one [128,1024] bank-pair
                xts = []
                for cp in range(NC // 2):
                    pt = bigp.tile([P, 2 * NB], f32, tag="big")
                    for ci in range(2):
                        c = 2 * cp + ci
                        for jj in range(NJ):
                            nc.tensor.transpose(
                                pt[:, ci * NB + jj * P : ci * NB + (jj + 1) * P],
                                xg(jj)[:, c * P : (c + 1) * P],
                                ident,
                            )
                    xt = xtp.tile([P, 2, NB], mmdt)
                    nc.vector.tensor_copy(xt, pt.rearrange("p (c n) -> p c n", c=2))
                    xts.append(xt)

                # Projections with concatenated stationary weights: one
                # M=128 pass computes two heads at once (M does not affect
                # matmul time). First half: [Wk|Wq] -> k at partitions 0:64
                # (kTp half A) and q at 64:128 (row-packed scores' B operand).
                # Second half: [Wv|Wk] -> k lands directly at partitions
                # 64:128 of kTp (no partition-shift DMA needed).
                w2 = wvk_sb if half2 else wkq_sb
                kq_ps = proj_psp.tile([P, NB], f32, tag="proj", name="kq_ps")
                for c in range(NC):
                    nc.tensor.matmul(
                        kq_ps,
                        w2[:, c, :],
                        xts[c // 2][:, c % 2, :],
                        start=(c == 0),
                        stop=(c == NC - 1),
                    )
                b2 = bvk_sb if half2 else bkq_sb
                if not half2:
                    # k rows 0:64 -> kTp half A; q rows 64:128 -> qTb
                    nc.scalar.activation(
                        kTp[j][0:H, :, :].rearrange("h j t -> h (j t)"),
                        kq_ps[0:H, :], AF.Relu, bias=b2[0:H, 0:1],
                    )
                    nc.scalar.activation(
                        qTb[j][H:P, :], kq_ps[H:P, :], AF.Relu,
                        bias=b2[H:P, 0:1],
                    )
                    nc.sync.dma_start(qTb[j][0:H, :], qTb[j][H:P, :])
                    # V in its own pass
                    v_ps = proj_psp.tile([H, NB], f32, tag="proj", name="v_ps")
                    for c in range(NC):
                        nc.tensor.matmul(
                            v_ps,
                            wv_sb[:, c, :],
                            xts[c // 2][:, c % 2, :],
                            start=(c == 0),
                            stop=(c == NC - 1),
                        )
                    v_sb = kvp.tile([H, NB], mmdt, tag="kvsb", name="v_sb")
                    nc.scalar.activation(v_sb, v_ps, AF.Relu, bias=bv_sb[:, 0:1])
                else:
                    # v rows 0:64; k rows 64:128 -> kTp half B directly
                    v_sb = kvp.tile([H, NB], mmdt, tag="kvsb", name="v_sb")
                    nc.scalar.activation(
                        v_sb, kq_ps[0:H, :], AF.Relu, bias=b2[0:H, 0:1]
                    )
                    nc.scalar.activation(
                        kTp[j][H:P, :, :].rearrange("h j t -> h (j t)"),
                        kq_ps[H:P, :], AF.Relu, bias=b2[H:P, 0:1],
                    )
                vt_ps = proj_psp.tile([P, NJ, H], mmdt, tag="proj", name="vt_ps")
                for jj in range(NJ):
                    nc.tensor.transpose(
                        vt_ps[:, jj, :],
                        v_sb[:, jj * P : (jj + 1) * P],
                        ident_r[0:H, 0:H],
                    )
                nc.vector.tensor_copy(Vg[g][:, :, 0:H], vt_ps)

            for j in range(NGH):
                do_group(j, after_dma=load_consts if j == 0 else None)
                do_group(j + NGH)

            # ---------------- attention ----------------
            scale = float(1.0 / np.sqrt(H))
            for qbp in range(NQB // 2):
                qbs = (2 * qbp, 2 * qbp + 1)
                o_ps = {
                    qb: o_psp.tile([H + 1, NB], f32, name=f"o_ps{qb}", tag="o_ps")
                    for qb in qbs
                }
                for p in range(NKP):
                    jg, i = p // NJ, p % NJ
                    e2 = {}
                    for qb in qbs:
                        s2 = bigp.tile([P, 2 * NB], f32, tag="big")
                        nc.tensor.matmul(
                            s2[:, 0:NB],
                            kTp[jg][0:H, i, :],
                            qTb[qb][0:H, :],
                            start=True,
                            stop=True,
                            tile_position=(0, 0),
                        )
                        nc.tensor.matmul(
                            s2[:, NB : 2 * NB],
                            kTp[jg][H:P, i, :],
                            qTb[qb][H:P, :],
                            start=True,
                            stop=True,
                            tile_position=(H, 0),
                        )
                        e = esbp.tile([P, 2 * NB], mmdt)
                        nc.scalar.activation(e, s2, AF.Exp, scale=scale)
                        e2[qb] = e
                    for qb in qbs:
                        nc.tensor.matmul(
                            o_ps[qb],
                            Vg[jg][:, i, :],
                            e2[qb][:, 0:NB],
                            start=(p == 0),
                            stop=False,
                        )
                        nc.tensor.matmul(
                            o_ps[qb],
                            Vg[NGH + jg][:, i, :],
                            e2[qb][:, NB : 2 * NB],
                            start=False,
                            stop=(p == NKP - 1),
                        )
                # normalize and store
                for qb in qbs:
                    o_sb = osbp.tile([H + 1, NB], f32)
                    nc.vector.tensor_copy(o_sb, o_ps[qb])
                    o4 = outp.tile([P, NJ, H], f32)
                    for jj in range(NJ):
                        ot = bigp.tile([P, H + 1], f32, tag="big")
                        nc.tensor.transpose(
                            ot,
                            o_sb[:, jj * P : (jj + 1) * P],
                            ident[0 : H + 1, 0 : H + 1],
                        )
                        recip = osbp.tile([P, 1], f32, tag="recip")
                        nc.vector.reciprocal(recip, ot[:, H : H + 1])
                        nc.vector.tensor_scalar_mul(o4[:, jj, :], ot[:, 0:H], recip)
                    q0 = qb * NB
                    nc.sync.dma_start(
                        out[q0 : q0 + NB, :].rearrange("(j p) h -> p j h", p=P), o4
                    )

    nc.compile()
    return nc


def _get_nc():
    if "nc" not in _cache:
        _cache["nc"] = _build(use_f32r=os.environ.get("K_NO_F32R", "") != "1")
    return _cache["nc"]


def _prep_inputs(x, Wk, bk, Wq, bq, Wv, bv):
    x = np.asarray(x, np.float32)
    wqt = np.asarray(Wq, np.float32).T
    wkt = np.asarray(Wk, np.float32).T
    wvt = np.ascontiguousarray(np.asarray(Wv, np.float32).T)
    wkq = np.ascontiguousarray(np.concatenate([wkt, wqt], axis=1))
    wvk = np.ascontiguousarray(np.concatenate([wvt, wkt], axis=1))
    bqc = np.asarray(bq, np.float32).reshape(H, 1)
    bkc = np.asarray(bk, np.float32).reshape(H, 1)
    bvc = np.asarray(bv, np.float32).reshape(H, 1)
    bkq = np.concatenate([bkc, bqc], axis=0)
    bvk = np.concatenate([bvc, bkc], axis=0)
    in_maps = []
    for i in range(NCORES):
        b, h = i // 2, i % 2
        xa = np.ascontiguousarray(x[b, h * TQ : (h + 1) * TQ])
        xbo = np.ascontiguousarray(x[b, (1 - h) * TQ : (2 - h) * TQ])
        in_maps.append(
            dict(xa=xa, xb=xbo, wkq=wkq, wvk=wvk, wvt=wvt,
                 bkq=bkq, bvk=bvk, bv=bvc)
        )
    return in_maps


def run(inputs, trace=False):
    from concourse.bass_utils import run_bass_kernel_spmd

    if not trace:
        # NTFF profiling is unavailable in this environment; make sure an
        # ambient BASS_TRACE can't divert the execute path.
        os.environ["BASS_NEVER_TRACE"] = "1"
    nc = _get_nc()
    in_maps = _prep_inputs(**inputs)
    res = run_bass_kernel_spmd(nc, in_maps, list(range(NCORES)), trace=trace)
    full = np.empty((B, T, H), np.float32)
    for i in range(NCORES):
        b, h = i // 2, i % 2
        full[b, h * TQ : (h + 1) * TQ] = res.results[i]["o"]
    return full, res


def kernel(**inputs):
    out, _ = run(inputs, trace=False)
    return out


# revision 20
# speedup vs baseline: 5.7656x; 1.0153x over previous
"""Trainium2 Bass kernel for nn_AttentionHead (B=4, T=4096, D=1024, H=64).

Sharding: 8 cores; core i handles (batch b = i//2, T-half = i%2): computes
attention output for its 2048 queries. K/V are computed per-core over the
full 4096 keys (weights tiny/replicated; key order is permutation-invariant
under softmax, so own-half-first ordering per core is fine).

Per-core dataflow (big matmuls in float32r = full-rate fp32 on the PE;
walrus requires fp32r operands to be produced *rounded*, so every matmul
input comes from a DVE copy or ACT activation with fp32r output dtype):
  - x tiles DMA'd contiguously, PE-transposed in fp32 to x^T staged in
    PSUM; the DVE copy to SBUF is the fp32->fp32r rounding point.
  - Projections use host-concatenated stationary weights so one M=128
    pass computes two heads at once (M does not affect matmul time):
    first T-half runs [Wk|Wq] (k -> kTp partitions 0:64, q -> 64:128,
    then DMA-duplicated down), second T-half runs [Wv|Wk] so k lands
    directly at partitions 64:128 of kTp. relu+bias on ACT writes the
    persistent fp32r tiles in place; scores then row-pack two
    64-contraction matmuls (row groups 0/64) per PSUM tile.
  - v_T PE-transposed to V natural [t,64]; column 64 = ones so attn@V also
    accumulates the softmax denominator.
  - scores s_T[k,q]: two k-tiles row-packed (contraction=64, row groups
    0/64) into one PSUM [128,1024] tile; exp on ACT with scale=1/8 (no max
    subtraction: scores are O(1) by construction). Two q-blocks are
    interleaved per k-pair so PE work hides the ACT exp chain.
  - attn@V: V'[128,65] stationary x exp[128,512] accumulated over 32
    k-tiles into PSUM [65,512]; row 64 = denominator. PE-transpose back,
    reciprocal*scale on DVE, DMA out.

Tensors are split at group granularity (kTp/Vg/qTb) so the Tile scheduler
can overlap the projection stage with attention as dependencies resolve.
"""

import os
import numpy as np

B, T, D, H = 4, 4096, 1024, 64
P = 128
NB = 512            # free-dim block size
TQ = T // 2         # queries per core
NCORES = 8

_cache = {}


def _build(use_f32r=True):
    import concourse.bass as bass
    import concourse.tile as tile
    from concourse import bacc, mybir
    from concourse.masks import make_identity

    f32 = mybir.dt.float32
    f32r = mybir.dt.float32r
    AF = mybir.ActivationFunctionType

    mmdt = f32r if use_f32r else f32

    nc = bacc.Bacc("TRN2", target_bir_lowering=False, debug=False)

    xa = nc.dram_tensor("xa", [TQ, D], mmdt, kind="ExternalInput").ap()
    xb = nc.dram_tensor("xb", [TQ, D], mmdt, kind="ExternalInput").ap()
    wkq = nc.dram_tensor("wkq", [D, P], f32, kind="ExternalInput").ap()
    wvk = nc.dram_tensor("wvk", [D, P], f32, kind="ExternalInput").ap()
    wvt = nc.dram_tensor("wvt", [D, H], f32, kind="ExternalInput").ap()
    bkq = nc.dram_tensor("bkq", [P, 1], f32, kind="ExternalInput").ap()
    bvk = nc.dram_tensor("bvk", [P, 1], f32, kind="ExternalInput").ap()
    bv = nc.dram_tensor("bv", [H, 1], f32, kind="ExternalInput").ap()
    out = nc.dram_tensor("o", [TQ, H], f32, kind="ExternalOutput").ap()

    NG = T // NB          # 8 K/V t-groups of 512
    NGH = NG // 2         # 4 groups per T-half
    NQB = TQ // NB        # 4 q-blocks of 512
    NKP = T // P // 2     # 16 k-tile pairs
    NC = D // P           # 8 d-chunks
    NJ = NB // P          # 4 t-subtiles per group

    with tile.TileContext(nc) as tc:
        with (
            tc.tile_pool(name="const", bufs=1) as constp,
            tc.tile_pool(name="persist", bufs=1) as persist,
            tc.tile_pool(name="xg", bufs=6) as xgp,
            tc.tile_pool(name="xt", bufs=8) as xtp,
            tc.tile_pool(name="kvsb", bufs=2) as kvp,
            tc.tile_pool(name="big_ps", bufs=2, space="PSUM") as bigp,
            tc.tile_pool(name="proj_ps", bufs=2, space="PSUM") as proj_psp,
            tc.tile_pool(name="o_ps", bufs=2, space="PSUM") as o_psp,
            tc.tile_pool(name="esb", bufs=4) as esbp,
            tc.tile_pool(name="osb", bufs=2) as osbp,
            tc.tile_pool(name="outp", bufs=2) as outp,
        ):
            ident = constp.tile([P, P], f32)
            make_identity(nc, ident)
            ident_r = constp.tile([P, P], mmdt)
            nc.vector.tensor_copy(ident_r, ident)
            wkq_ld = constp.tile([P, NC, P], f32)
            wvk_ld = constp.tile([P, NC, P], f32)
            wv_ld = constp.tile([P, NC, H], f32)
            wkq_sb = constp.tile([P, NC, P], mmdt)
            wvk_sb = constp.tile([P, NC, P], mmdt)
            wv_sb = constp.tile([P, NC, H], mmdt)
            bkq_sb = constp.tile([P, 1], f32)
            bvk_sb = constp.tile([P, 1], f32)
            bv_sb = constp.tile([H, 1], f32)

            def load_consts():
                # emitted after the first x-block DMA so the transposes (the
                # first PE work) aren't stuck behind the weight loads; DVE
                # copies round fp32 -> fp32r (walrus requires rounded inputs)
                nc.sync.dma_start(wkq_ld, wkq.rearrange("(c p) h -> p c h", p=P))
                nc.sync.dma_start(wvk_ld, wvk.rearrange("(c p) h -> p c h", p=P))
                nc.sync.dma_start(wv_ld, wvt.rearrange("(c p) h -> p c h", p=P))
                nc.vector.tensor_copy(wkq_sb, wkq_ld)
                nc.vector.tensor_copy(wvk_sb, wvk_ld)
                nc.vector.tensor_copy(wv_sb, wv_ld)
                nc.sync.dma_start(bkq_sb, bkq)
                nc.sync.dma_start(bvk_sb, bvk)
                nc.sync.dma_start(bv_sb, bv)

            # persistent attention operands, split per group for overlap
            kTp = [persist.tile([P, NJ, P], mmdt, name=f"kTp{j}") for j in range(NGH)]
            qTb = [persist.tile([P, NB], mmdt, name=f"qTb{j}") for j in range(NQB)]
            Vg = [persist.tile([P, NJ, H + 1], mmdt, name=f"Vg{g}") for g in range(NG)]
            onesc = constp.tile([P, NJ, 1], f32)
            nc.gpsimd.memset(onesc, 1.0)
            for g in range(NG):
                nc.vector.tensor_copy(Vg[g][:, :, H : H + 1], onesc)

            # ---------------- projections ----------------
            def do_group(g, after_dma=None):
                half2 = g >= NGH          # second T-half (keys 2048..4095)
                src = xb if half2 else xa
                j = g % NGH
                r0 = j * NB
                # two half-loads so transposes start after the first lands
                xga = xgp.tile([P, NJ // 2, D], mmdt, tag="xg")
                xgb = xgp.tile([P, NJ // 2, D], mmdt, tag="xg")
                nc.sync.dma_start(
                    xga,
                    src[r0 : r0 + NB // 2, :].rearrange("(j p) d -> p j d", p=P),
                )
                nc.sync.dma_start(
                    xgb,
                    src[r0 + NB // 2 : r0 + NB, :].rearrange("(j p) d -> p j d", p=P),
                )
                if after_dma is not None:
                    after_dma()

                def xg(jj):
                    return (xga if jj < NJ // 2 else xgb)[:, jj % (NJ // 2), :]
                # transpose to x^T: c-pairs staged through one [128,1024] bank-pair
                xts = []
                for cp in range(NC // 2):
                    pt = bigp.tile([P, 2 * NB], mmdt, tag="big")
                    for ci in range(2):
                        c = 2 * cp + ci
                        for jj in range(NJ):
                            nc.tensor.transpose(
                                pt[:, ci * NB + jj * P : ci * NB + (jj + 1) * P],
                                xg(jj)[:, c * P : (c + 1) * P],
                                ident_r,
                            )
                    xt = xtp.tile([P, 2, NB], mmdt)
                    nc.vector.tensor_copy(xt, pt.rearrange("p (c n) -> p c n", c=2))
                    xts.append(xt)

                # Projections with concatenated stationary weights: one
                # M=128 pass computes two heads at once (M does not affect
                # matmul time). First half: [Wk|Wq] -> k at partitions 0:64
                # (kTp half A) and q at 64:128 (row-packed scores' B operand).
                # Second half: [Wv|Wk] -> k lands directly at partitions
                # 64:128 of kTp (no partition-shift DMA needed).
                w2 = wvk_sb if half2 else wkq_sb
                kq_ps = proj_psp.tile([P, NB], f32, tag="proj", name="kq_ps")
                for c in range(NC):
                    nc.tensor.matmul(
                        kq_ps,
                        w2[:, c, :],
                        xts[c // 2][:, c % 2, :],
                        start=(c == 0),
                        stop=(c == NC - 1),
                    )
                b2 = bvk_sb if half2 else bkq_sb
                if not half2:
                    # k rows 0:64 -> kTp half A; q rows 64:128 -> qTb
                    nc.scalar.activation(
                        kTp[j][0:H, :, :].rearrange("h j t -> h (j t)"),
                        kq_ps[0:H, :], AF.Relu, bias=b2[0:H, 0:1],
                    )
                    nc.scalar.activation(
                        qTb[j][H:P, :], kq_ps[H:P, :], AF.Relu,
                        bias=b2[H:P, 0:1],
                    )
                    nc.sync.dma_start(qTb[j][0:H, :], qTb[j][H:P, :])
                    # V in its own pass
                    v_ps = proj_psp.tile([H, NB], f32, tag="proj", name="v_ps")
                    for c in range(NC):
                        nc.tensor.matmul(
                            v_ps,
                            wv_sb[:, c, :],
                            xts[c // 2][:, c % 2, :],
                            start=(c == 0),
                            stop=(c == NC - 1),
                        )
                    v_sb = kvp.tile([H, NB], mmdt, tag="kvsb", name="v_sb")
                    nc.scalar.activation(v_sb, v_ps, AF.Relu, bias=bv_sb[:, 0:1])
                else:
                    # v rows 0:64; k rows 64:128 -> kTp half B directly
                    v_sb = kvp.tile([H, NB], mmdt, tag="kvsb", name="v_sb")
                    nc.scalar.activation(
                        v_sb, kq_ps[0:H, :], AF.Relu, bias=b2[0:H, 0:1]
                    )
                    nc.scalar.activation(
                        kTp[j][H:P, :, :].rearrange("h j t -> h (j t)"),
                        kq_ps[H:P, :], AF.Relu, bias=b2[H:P, 0:1],
                    )
                vt_ps = proj_psp.tile([P, NJ, H], mmdt, tag="proj", name="vt_ps")
                for jj in range(NJ):
                    nc.tensor.transpose(
                        vt_ps[:, jj, :],
                        v_sb[:, jj * P : (jj + 1) * P],
                        ident_r[0:H, 0:H],
                    )
                nc.vector.tensor_copy(Vg[g][:, :, 0:H], vt_ps)

            for j in range(NGH):
                do_group(j, after_dma=load_consts if j == 0 else None)
                do_group(j + NGH)

            # ---------------- attention ----------------
            scale = float(1.0 / np.sqrt(H))
            for qbp in range(NQB // 2):
                qbs = (2 * qbp, 2 * qbp + 1)
                o_ps = {
                    qb: o_psp.tile([H + 1, NB], f32, name=f"o_ps{qb}", tag="o_ps")
                    for qb in qbs
                }
                for p in range(NKP):
                    jg, i = p // NJ, p % NJ
                    e2 = {}
                    for qb in qbs:
                        s2 = bigp.tile([P, 2 * NB], f32, tag="big")
                        nc.tensor.matmul(
                            s2[:, 0:NB],
                            kTp[jg][0:H, i, :],
                            qTb[qb][0:H, :],
                            start=True,
                            stop=True,
                            tile_position=(0, 0),
                        )
                        nc.tensor.matmul(
                            s2[:, NB : 2 * NB],
                            kTp[jg][H:P, i, :],
                            qTb[qb][H:P, :],
                            start=True,
                            stop=True,
                            tile_position=(H, 0),
                        )
                        e = esbp.tile([P, 2 * NB], mmdt)
                        nc.scalar.activation(e, s2, AF.Exp, scale=scale)
                        e2[qb] = e
                    for qb in qbs:
                        nc.tensor.matmul(
                            o_ps[qb],
                            Vg[jg][:, i, :],
                            e2[qb][:, 0:NB],
                            start=(p == 0),
                            stop=False,
                        )
                        nc.tensor.matmul(
                            o_ps[qb],
                            Vg[NGH + jg][:, i, :],
                            e2[qb][:, NB : 2 * NB],
                            start=False,
                            stop=(p == NKP - 1),
                        )
                # normalize and store
                for qb in qbs:
                    o_sb = osbp.tile([H + 1, NB], f32)
                    nc.vector.tensor_copy(o_sb, o_ps[qb])
                    o4 = outp.tile([P, NJ, H], f32)
                    for jj in range(NJ):
                        ot = bigp.tile([P, H + 1], f32, tag="big")
                        nc.tensor.transpose(
                            ot,
                            o_sb[:, jj * P : (jj + 1) * P],
                            ident[0 : H + 1, 0 : H + 1],
                        )
                        recip = osbp.tile([P, 1], f32, tag="recip")
                        nc.vector.reciprocal(recip, ot[:, H : H + 1])
                        nc.vector.tensor_scalar_mul(o4[:, jj, :], ot[:, 0:H], recip)
                    q0 = qb * NB
                    nc.sync.dma_start(
                        out[q0 : q0 + NB, :].rearrange("(j p) h -> p j h", p=P), o4
                    )

    nc.compile()
    return nc


def _get_nc():
    if "nc" not in _cache:
        _cache["nc"] = _build(use_f32r=os.environ.get("K_NO_F32R", "") != "1")
    return _cache["nc"]


def _prep_inputs(x, Wk, bk, Wq, bq, Wv, bv):
    x = np.asarray(x, np.float32)
    wqt = np.asarray(Wq, np.float32).T
    wkt = np.asarray(Wk, np.float32).T
    wvt = np.ascontiguousarray(np.asarray(Wv, np.float32).T)
    wkq = np.ascontiguousarray(np.concatenate([wkt, wqt], axis=1))
    wvk = np.ascontiguousarray(np.concatenate([wvt, wkt], axis=1))
    bqc = np.asarray(bq, np.float32).reshape(H, 1)
    bkc = np.asarray(bk, np.float32).reshape(H, 1)
    bvc = np.asarray(bv, np.float32).reshape(H, 1)
    bkq = np.concatenate([bkc, bqc], axis=0)
    bvk = np.concatenate([bvc, bkc], axis=0)
    in_maps = []
    for i in range(NCORES):
        b, h = i // 2, i % 2
        xa = np.ascontiguousarray(x[b, h * TQ : (h + 1) * TQ])
        xbo = np.ascontiguousarray(x[b, (1 - h) * TQ : (2 - h) * TQ])
        in_maps.append(
            dict(xa=xa, xb=xbo, wkq=wkq, wvk=wvk, wvt=wvt,
                 bkq=bkq, bvk=bvk, bv=bvc)
        )
    return in_maps


def run(inputs, trace=False):
    from concourse.bass_utils import run_bass_kernel_spmd

    if not trace:
        # NTFF profiling is unavailable in this environment; make sure an
        # ambient BASS_TRACE can't divert the execute path.
        os.environ["BASS_NEVER_TRACE"] = "1"
    nc = _get_nc()
    in_maps = _prep_inputs(**inputs)
    res = run_bass_kernel_spmd(nc, in_maps, list(range(NCORES)), trace=trace)
    full = np.empty((B, T, H), np.float32)
    for i in range(NCORES):
        b, h = i // 2, i % 2
        full[b, h * TQ : (h + 1) * TQ] = res.results[i]["o"]
    return full, res


def kernel(**inputs):
    out, _ = run(inputs, trace=False)
    return out
